# revision 1
# baseline (speedup 1.0000x reference)
"""Trainium2 Bass kernel for nn_Disp_61125974557155.

Computes: trilinear upsample of a cost volume [B,1,48,64,128] ->
[B,193,256,512] (align_corners=False, edge-replicated), softmin over
disparity, disparity regression -> [B,256,512].

Design (per core; 8 cores = 2 batches x 4 H'-quarters):
  - Host: edge-pad x (replicate), slice the core's H-halo shard, and stack a
    copy shifted by one h-row on partitions 50..99 (sharding/memory movement
    only, no arithmetic).
  - DVE: W-axis 4x lerp at low resolution -> xsw [100, 19, 4, 128] f32r
    (rw-major planes; w' = 4*s + rw), fine-grained row groups so the PE
    pipeline starts as soon as the first rows land.
  - PE: D-expansion (48->193 banded lerp matrix) with the H-axis 4x lerp
    FOLDED into the same matmul via the dup-shifted operand halves:
    vol = A2r^T @ xsw_row -> PSUM [d'-chunk, 512] tiles, float32r.
  - ACT: e = exp(-vol) (PSUM -> SBUF, bf16), batched 3 tiles per instr.
  - PE: S0 = sum_d e, S1 = sum_d d*e as "flipped" bf16 matmuls (e-slice
    stationary [K,128], R moving [K,2]) -> pixel-major [128,2] stats packed
    along the free axis of one persistent PSUM bank. PSUM has_written is
    per-element, so only the very first stat matmul starts the group;
    chunk-1 slots overwrite-where-clear, chunk-2 accumulates.
  - PE matmuls are chained in emission order (same-dtype bursts): the
    scheduler otherwise interleaves bf16 stat matmuls into the f32r vol
    runs, paying a large FP32-HIGH <-> bf16 mode-switch penalty.
  - DVE: out = S1 * recip(S0); PE transposes q-planes via identity; DMA out.
"""

import numpy as np
from contextlib import ExitStack

import concourse.bass as bass
import concourse.bacc as bacc
import concourse.tile as tile
from concourse import mybir
from concourse.bass_utils import run_bass_kernel_spmd
from concourse.tile_rust import add_dep_helper

F32 = mybir.dt.float32
F32R = mybir.dt.float32r
BF16 = mybir.dt.bfloat16

MAXDISP = 192
DP = MAXDISP + 1      # 193 disparities
KD = 48               # low-res D
KP = KD + 2           # padded k' (edge-replicated)
NCORES = 8
WH = (0.625, 0.875, 0.125, 0.375)   # H lerp fracs per r = h' % 4
WW = (0.625, 0.875, 0.125, 0.375)   # W lerp fracs per rw = w' % 4
CHUNKS = ((0, 128), (128, 65))      # d' chunk (offset, size)
NROW = 19                            # h-rows in dup-packed shard
ROW_GROUPS = ((0, 1), (1, 1), (2, 2), (4, 4), (8, 4), (12, 4), (16, 3))
TGROUPS = tuple((2 * i, 2) for i in range(8))  # t-tiles per pv


def _build_ad() -> np.ndarray:
    """A_D [193, 50]: D-axis linear upsample matrix on padded k' = k+1."""
    ad = np.zeros((DP, KP), dtype=np.float64)
    for dp in range(DP):
        i = (dp + 0.5) * KD / DP - 0.5
        fl = int(np.floor(i))
        fr = i - fl
        ad[dp, fl + 1] += 1.0 - fr
        ad[dp, fl + 2] += fr
    return ad


def _build_consts():
    ad = _build_ad()                      # [193, 50]
    amat = np.zeros((2 * KP, 4, DP), dtype=np.float64)
    for r in range(4):
        amat[:KP, r, :] = (1.0 - WH[r]) * ad.T
        amat[KP:, r, :] = WH[r] * ad.T
    rmat = np.zeros((128, 4), dtype=np.float64)
    rmat[:, 0] = 1.0
    rmat[:, 1] = np.arange(128)
    rmat[: DP - 128, 2] = 1.0
    rmat[: DP - 128, 3] = np.arange(128, DP)
    ident = np.eye(128, dtype=np.float32)
    return (
        np.ascontiguousarray(amat.reshape(2 * KP, 4 * DP), dtype=np.float32),
        rmat.astype(np.float32).astype(mybir.dt.np(BF16)),
        ident,
    )


def _build_nc() -> bass.Bass:
    nc = bacc.Bacc()
    xsd = nc.declare_dram_parameter("xsd", [2 * KP, NROW * 130], F32, isOutput=False)
    amat = nc.declare_dram_parameter("amat", [2 * KP, 4 * DP], F32R, isOutput=False)
    rmat = nc.declare_dram_parameter("rmat", [128, 4], BF16, isOutput=False)
    ident = nc.declare_dram_parameter("ident", [128, 128], F32, isOutput=False)
    outp = nc.declare_dram_parameter("out", [64, 512], F32, isOutput=True)

    xsd_v = xsd.rearrange("p (h w) -> p h w", h=NROW)
    amat_v = amat.rearrange("p (r d) -> p r d", r=4)

    mult = mybir.AluOpType.mult
    add = mybir.AluOpType.add
    exp_fn = mybir.ActivationFunctionType.Exp

    # Chain PE matmuls in emission order: the Tile scheduler otherwise
    # interleaves ready bf16 stat matmuls into the f32r vol runs, paying the
    # PE FP32-HIGH <-> bf16 mode-switch penalty on nearly every instruction.
    last_pe = [None]

    def pe_matmul(*args, **kwargs):
        ins = nc.tensor.matmul(*args, **kwargs)
        if last_pe[0] is not None:
            add_dep_helper(ins.ins, last_pe[0].ins, False,
                           "keep same-dtype matmul bursts contiguous")
        last_pe[0] = ins
        return ins

    with ExitStack() as ctx:
        tc = ctx.enter_context(tile.TileContext(nc))
        singles = ctx.enter_context(tc.tile_pool(name="singles", bufs=1))
        tmp_pool = ctx.enter_context(tc.tile_pool(name="tmp", bufs=4))
        epool = ctx.enter_context(tc.tile_pool(name="epool", bufs=20))
        fin = ctx.enter_context(tc.tile_pool(name="fin", bufs=1))
        pvol = ctx.enter_context(tc.tile_pool(name="pvol", bufs=3, space="PSUM"))
        pstat = ctx.enter_context(tc.tile_pool(name="pstat", bufs=1, space="PSUM"))
        ptr = ctx.enter_context(tc.tile_pool(name="ptr", bufs=1, space="PSUM"))

        # ---- input loads: xsd first (gates the lerp chain) on the sync
        # HWDGE queue; constants go through gpsimd SWDGE in parallel ----
        s_xsd = []
        for g, (g0, gn) in enumerate(ROW_GROUPS):
            t_x = singles.tile([2 * KP, gn, 130], F32, tag=f"xsd{g}")
            nc.sync.dma_start(out=t_x, in_=xsd_v[:, g0 : g0 + gn, :])
            s_xsd.append(t_x)
        s_am = {}
        for ci, (d0, dn) in enumerate(CHUNKS):
            for r in range(4):
                t_a = singles.tile([2 * KP, dn], F32R, tag=f"am{ci}{r}")
                nc.gpsimd.dma_start(out=t_a, in_=amat_v[:, r, d0 : d0 + dn])
                s_am[(ci, r)] = t_a
        s_rm = singles.tile([128, 4], BF16, tag="rm")
        nc.gpsimd.dma_start(out=s_rm, in_=rmat[:, :])
        s_id = singles.tile([128, 128], F32, tag="id")
        nc.gpsimd.dma_start(out=s_id, in_=ident[:, :])

        # ---- W-axis 4x lerp at low res, rw-major planes ----
        # xsw[p, h, rw, s] = (1-fr)*xs[p, h, c+s] + fr*xs[p, h, c+s+1]
        # ACT (idle at startup) computes fr*hi into scratch; DVE fuses.
        # Per group: one shared difference d[s] = xs[s] - xs[s+1], then each
        # rw plane is a single fused op hi + c*d (the rw pairs share source
        # windows and d for the c=1 pair is just d shifted by one column):
        #   rw0 = xs[s+1] + 0.375*d[s]    rw1 = xs[s+1] + 0.125*d[s]
        #   rw2 = xs[s+2] + 0.875*d[s+1]  rw3 = xs[s+2] + 0.625*d[s+1]
        s_xsw = []
        for g, (g0, gn) in enumerate(ROW_GROUPS):
            t_w = singles.tile([2 * KP, gn, 4, 128], F32R, tag=f"xsw{g}")
            t_d = tmp_pool.tile([2 * KP, gn, 129], F32, tag="wld")
            nc.vector.tensor_sub(
                t_d, s_xsd[g][:, :, 0:129], s_xsd[g][:, :, 1:130]
            )
            for rw, (coef, dc, hc) in enumerate(
                ((0.375, 0, 1), (0.125, 0, 1), (0.875, 1, 2), (0.625, 1, 2))
            ):
                nc.vector.scalar_tensor_tensor(
                    out=t_w[:, :, rw, :],
                    in0=t_d[:, :, dc : dc + 128],
                    scalar=coef,
                    in1=s_xsd[g][:, :, hc : hc + 128],
                    op0=mult,
                    op1=add,
                )
            s_xsw.append(t_w)

        def xsw_row(l: int) -> bass.AP:
            for g, (g0, gn) in enumerate(ROW_GROUPS):
                if g0 <= l < g0 + gn:
                    return s_xsw[g][:, l - g0, :, :]
            raise IndexError(l)

        # ---- persistent pixel-major stats bank ----
        # free layout [q(4), j(64), s01(2)]: ps[p, q, j, :] = (S0, S1) of
        # output pixel (h'-row j, w' = 4*p + q)
        ps = pstat.tile([128, 512], F32, tag="ps")
        ps_v = ps.rearrange("p (q j s) -> p q j s", q=4, s=2)

        # ---- main passes: chunk-major, software-pipelined by one phase ----
        # Flip batches of phase i-1 are interleaved after each vol t-group of
        # phase i so ACT never starves at phase boundaries. The first phase
        # uses 1-t groups so the pipeline fills as soon as the first xsw rows
        # exist.
        phases = [(ci, r) for ci, _ in enumerate(CHUNKS) for r in range(4)]
        FINE = ((0, 1), (1, 1)) + TGROUPS[1:]

        def flip_batch(ci, r, et, gt0, gtn, first, q_of_t=None):
            dn = CHUNKS[ci][1]
            rhs_red = s_rm[0:dn, 2 * ci : 2 * ci + 2]               # [dn, 2] bf16
            for ut in range(gtn):
                t = gt0 + ut
                j = 4 * t + r
                for q in range(4):
                    # PSUM has_written is per-element: only the very first
                    # stat matmul starts the group (clears the whole bank's
                    # bits); chunk-1 slots then overwrite-where-clear,
                    # chunk-2 accumulates.
                    first_mm = first and t == 0 and q == 0
                    pe_matmul(
                        ps_v[:, q, j, :],
                        et[0:dn, 512 * ut + 128 * q : 512 * ut + 128 * (q + 1)],
                        rhs_red,
                        start=first_mm,
                        stop=(ci == 1 and r == 3 and t == 15 and q == 3),
                        skip_group_check=True,
                    )

        prev_batches = []   # (ci, r, et, gt0, gtn, first) flip batches
        for pi, (ci, r) in enumerate(phases):
            d0, dn = CHUNKS[ci]
            lhsT_vol = s_am[(ci, r)][:, :]                          # [100, dn] f32r
            tgroups = FINE if pi == 0 else TGROUPS
            cur = []
            k = 0
            for gt0, gtn in tgroups:
                pv = pvol.tile([128, 2 * 512], F32, tag="pv")
                et = epool.tile([128, 2 * 512], BF16, tag="e")
                for ut in range(gtn):
                    t = gt0 + ut
                    l = t if r < 2 else t + 1
                    rhs = xsw_row(l).rearrange("p q s -> p (q s)")  # [100, 512] f32r
                    pe_matmul(
                        pv[0:dn, 512 * ut : 512 * (ut + 1)],
                        lhsT_vol,
                        rhs,
                        start=True,
                        stop=True,
                    )
                nc.scalar.activation(
                    et[0:dn, 0 : 512 * gtn], pv[0:dn, 0 : 512 * gtn],
                    exp_fn, scale=-1.0,
                )
                # interleave one flip batch from the previous phase
                if k < len(prev_batches):
                    flip_batch(*prev_batches[k])
                    k += 1
                cur.append((ci, r, et, gt0, gtn, (ci, r) == (0, 0)))
            while k < len(prev_batches):
                flip_batch(*prev_batches[k])
                k += 1
            prev_batches = cur
        for fb in prev_batches:
            flip_batch(*fb)

        # ---- finalize: out = S1 * recip(S0); transpose q-planes ----
        # oo[p=s, q, j] = out pixel (h'-row j, w' = 4*s + q); per-q chain so
        # the transposes/copies pipeline behind the reciprocal.
        rec = fin.tile([128, 4, 64], F32, tag="rec")
        oo = fin.tile([128, 4, 64], F32, tag="oo")
        om = fin.tile([64, 128, 4], F32, tag="om")
        for q in range(4):
            nc.vector.reciprocal(rec[:, q, :], ps_v[:, q, :, 0])
            nc.vector.tensor_mul(oo[:, q, :], ps_v[:, q, :, 1], rec[:, q, :])
            tr = ptr.tile([64, 128], F32, tag="tr")
            nc.tensor.transpose(tr, oo[:, q, :], s_id)
            nc.vector.tensor_copy(om[:, :, q], tr)
        nc.sync.dma_start(out=outp[:, :], in_=om.rearrange("j s q -> j (s q)"))

    nc.compile()
    return nc


_CACHE: dict = {}


def _shard_inputs(x: np.ndarray):
    """Edge-pad and slice per-core shards (memory movement only)."""
    xpad = np.pad(x[:, 0], ((0, 0), (1, 1), (1, 3), (1, 1)), mode="edge")
    amat, rmat, ident = _build_consts()
    in_maps = []
    for c in range(NCORES):
        b, q = divmod(c, 4)
        xs = xpad[b][:, 16 * q : 16 * q + 20, :]          # [50, 20, 130]
        xsd = np.concatenate([xs[:, 0:19, :], xs[:, 1:20, :]], axis=0)
        xsd = np.ascontiguousarray(
            xsd.reshape(2 * KP, NROW * 130), dtype=np.float32
        )
        in_maps.append({"xsd": xsd, "amat": amat, "rmat": rmat, "ident": ident})
    return in_maps


def kernel(x: np.ndarray, _trace: bool = False, _tmpdir=None):
    x = np.asarray(x, dtype=np.float32)
    assert x.shape == (2, 1, 48, 64, 128), x.shape
    if "nc" not in _CACHE:
        _CACHE["nc"] = _build_nc()
    nc = _CACHE["nc"]
    in_maps = _shard_inputs(x)
    res = run_bass_kernel_spmd(
        nc, in_maps, list(range(NCORES)), trace=_trace, tmpdir=_tmpdir
    )
    out = np.zeros((2, 256, 512), dtype=np.float32)
    for c in range(NCORES):
        b, q = divmod(c, 4)
        out[b, 64 * q : 64 * (q + 1), :] = res.results[c]["out"]
    if _trace:
        return out, res
    return out

